# revision 6
# baseline (speedup 1.0000x reference)
"""MinGRU Trainium2 kernel (nn_MinGRU_60421599920446).

Math (per batch row b):
    vz[s,h] = x[s,:] @ w_z^T + bz      vh[s,h] = x[s,:] @ w_h^T + bh
    z = sigmoid(vz); h_t = (1-z_t)*h_{t-1} + z_t*vh_t   (scan over s)

Strategy: data-parallel over batch, 1 row per NeuronCore (8 cores).
Per core, work in the transposed domain [H on partitions, S on free] so the
recurrence maps onto the DVE `tensor_tensor_scan` instruction (one affine
scan per partition lane along the free axis):
    state = a_t * state + b_t   with  a = 1-z = sigmoid(-(vz+bz)),
                                      b = z * (vh+bh)

Pipeline per 1024-step s-chunk:
    DMA x chunk in (natural [s,d] layout) ->
    PE 128x128 transposes -> ACT copies PSUM->SBUF (x^T) ->
    PE fp32 matmuls (W^T stationary) -> vz/vh in PSUM ->
    ACT: z = Sigmoid(vz+bz), a = Sigmoid(-vz-bz)  (both read PSUM) ->
    DVE: b = (vh + bh) * z  (scalar_tensor_tensor, reads PSUM) ->
    DVE: tensor_tensor_scan (carry chained across chunks via last column) ->
    PE transposes h back to natural [s,h] -> DVE copies PSUM->SBUF ->
    DMA out.
"""

import numpy as np
from contextlib import ExitStack

B, S, D, H = 8, 8192, 256, 256
N_CORES = 8

_CACHE = {}


def _build(seq_len, chunk):
    """Build + compile the single-core SPMD Bass program."""
    import concourse.bacc as bacc
    import concourse.tile as tile
    import concourse.mybir as mybir

    dt = mybir.dt
    f32 = dt.float32
    AF = mybir.ActivationFunctionType
    OP = mybir.AluOpType

    assert chunk % 512 == 0 and seq_len % chunk == 0
    nblk = chunk // 128          # 128-row blocks per chunk
    nchunk = seq_len // chunk

    nc = bacc.Bacc("TRN2", target_bir_lowering=False, debug=False)

    x_d = nc.dram_tensor("x", [seq_len, D], f32, kind="ExternalInput").ap()
    wzT_d = nc.dram_tensor("wzT", [D, H], f32, kind="ExternalInput").ap()
    whT_d = nc.dram_tensor("whT", [D, H], f32, kind="ExternalInput").ap()
    # packed per-partition columns: [half m][128][h0, bz, -bz, bh]
    cols_d = nc.dram_tensor("cols", [2, 128, 4], f32, kind="ExternalInput").ap()
    id_d = nc.dram_tensor("ident", [128, 128], f32, kind="ExternalInput").ap()
    out_d = nc.dram_tensor("out", [seq_len, H], f32, kind="ExternalOutput").ap()

    # chunked views: [chunk-idx, partition(s within block), block, feature]
    x_v = x_d.rearrange("(c t p) d -> c p t d", p=128, t=nblk)
    out_v = out_d.rearrange("(c t p) h -> c p t h", p=128, t=nblk)

    with tile.TileContext(nc) as tc, ExitStack() as ctx:
        const = ctx.enter_context(tc.tile_pool(name="const", bufs=1))
        xin = ctx.enter_context(tc.tile_pool(name="xin", bufs=2))
        xTp = ctx.enter_context(tc.tile_pool(name="xT", bufs=2))
        zp = ctx.enter_context(tc.tile_pool(name="z", bufs=2))
        ap_ = ctx.enter_context(tc.tile_pool(name="a", bufs=2))
        bp = ctx.enter_context(tc.tile_pool(name="b", bufs=2))
        hp = ctx.enter_context(tc.tile_pool(name="h", bufs=2))
        hop = ctx.enter_context(tc.tile_pool(name="ho", bufs=2))
        vzp = ctx.enter_context(tc.tile_pool(name="vz", bufs=2, space="PSUM"))
        vhp = ctx.enter_context(tc.tile_pool(name="vh", bufs=2, space="PSUM"))
        trp = ctx.enter_context(tc.tile_pool(name="tr", bufs=2, space="PSUM"))

        ident = const.tile([128, 128], f32, tag="ident")
        nc.sync.dma_start(ident[:], id_d[:, :])
        cols = []
        for m in range(2):
            t = const.tile([128, 4], f32, tag=f"cols{m}")
            nc.sync.dma_start(t[:], cols_d[m])
            cols.append(t)
        wzT, whT = [], []
        for k in range(2):
            tz = const.tile([128, H], f32, tag=f"wz{k}")
            nc.sync.dma_start(tz[:], wzT_d[k * 128:(k + 1) * 128, :])
            wzT.append(tz)
            th = const.tile([128, H], f32, tag=f"wh{k}")
            nc.sync.dma_start(th[:], whT_d[k * 128:(k + 1) * 128, :])
            whT.append(th)

        h_prev = None
        for c in range(nchunk):
            xn = xin.tile([128, nblk * D], f32, tag="xn")
            nc.sync.dma_start(xn[:].rearrange("p (t d) -> p t d", d=D), x_v[c])

            # ---- transpose x chunk: [s,d] blocks -> xT[k] = [d-half, s] ----
            xT = [xTp.tile([128, chunk], f32, tag=f"xt{k}", name=f"xt{k}")
                  for k in range(2)]
            for k in range(2):
                for g in range(chunk // 512):
                    pt = trp.tile([128, 512], f32, tag="tr")
                    for j in range(4):
                        t = g * 4 + j
                        nc.tensor.transpose(
                            pt[:, j * 128:(j + 1) * 128],
                            xn[:, t * D + k * 128: t * D + (k + 1) * 128],
                            ident[:],
                        )
                    nc.scalar.copy(xT[k][:, g * 512:(g + 1) * 512], pt[:])

            # ---- z-projection matmuls, then z / a on ACT ----
            vz = [vzp.tile([128, chunk], f32, tag="vz", name=f"vz{m}")
                  for m in range(2)]
            for m in range(2):
                for k in range(2):
                    for s2 in range(chunk // 512):
                        nc.tensor.matmul(
                            vz[m][:, s2 * 512:(s2 + 1) * 512],
                            wzT[k][:, m * 128:(m + 1) * 128],
                            xT[k][:, s2 * 512:(s2 + 1) * 512],
                            start=(k == 0), stop=(k == 1),
                        )
            z = [zp.tile([128, chunk], f32, tag=f"z{m}", name=f"z{m}") for m in range(2)]
            a = [ap_.tile([128, chunk], f32, tag=f"a{m}", name=f"a{m}") for m in range(2)]
            for m in range(2):
                nc.scalar.activation(z[m][:], vz[m][:], AF.Sigmoid,
                                     bias=cols[m][:, 1:2], scale=1.0)
                nc.scalar.activation(a[m][:], vz[m][:], AF.Sigmoid,
                                     bias=cols[m][:, 2:3], scale=-1.0)

            # ---- h-projection matmuls + b = (vh + bh) * z on DVE ----
            b = [bp.tile([128, chunk], f32, tag=f"b{m}", name=f"b{m}") for m in range(2)]
            for m in range(2):
                for s2 in range(chunk // 512):
                    vht = vhp.tile([128, 512], f32, tag="vh")
                    for k in range(2):
                        nc.tensor.matmul(
                            vht[:],
                            whT[k][:, m * 128:(m + 1) * 128],
                            xT[k][:, s2 * 512:(s2 + 1) * 512],
                            start=(k == 0), stop=(k == 1),
                        )
                    nc.vector.scalar_tensor_tensor(
                        b[m][:, s2 * 512:(s2 + 1) * 512],
                        vht[:], cols[m][:, 3:4],
                        z[m][:, s2 * 512:(s2 + 1) * 512],
                        op0=OP.add, op1=OP.mult,
                    )

            # ---- the recurrence: DVE affine scan along s ----
            h = [hp.tile([128, chunk], f32, tag=f"h{m}", name=f"h{m}") for m in range(2)]
            for m in range(2):
                init = (cols[m][:, 0:1] if c == 0
                        else h_prev[m][:, chunk - 1:chunk])
                nc.vector.tensor_tensor_scan(
                    h[m][:], a[m][:], b[m][:], init,
                    op0=OP.mult, op1=OP.add,
                )
            h_prev = h

            # ---- transpose h back to natural [s, h] and store ----
            ho = hop.tile([128, nblk * H], f32, tag="ho")
            for g in range(chunk // 256):
                pt = trp.tile([128, 512], f32, tag="tr")
                for j in range(2):
                    t = g * 2 + j
                    for m in range(2):
                        nc.tensor.transpose(
                            pt[:, j * 256 + m * 128: j * 256 + (m + 1) * 128],
                            h[m][:, t * 128:(t + 1) * 128],
                            ident[:],
                        )
                nc.vector.tensor_copy(ho[:, g * 512:(g + 1) * 512], pt[:])
            nc.sync.dma_start(out_v[c],
                              ho[:].rearrange("p (t h) -> p t h", h=H))

    nc.compile()
    return nc


def _get(seq_len, chunk):
    key = (seq_len, chunk)
    if key not in _CACHE:
        _CACHE[key] = _build(seq_len, chunk)
    return _CACHE[key]


def _make_in_maps(x, h0, w_h_w, w_h_b, w_z_w, w_z_b, n_cores=N_CORES):
    wzT = np.ascontiguousarray(np.asarray(w_z_w, np.float32).T)
    whT = np.ascontiguousarray(np.asarray(w_h_w, np.float32).T)
    bz = np.asarray(w_z_b, np.float32).reshape(2, 128)
    bh = np.asarray(w_h_b, np.float32).reshape(2, 128)
    ident = np.eye(128, dtype=np.float32)
    in_maps = []
    for i in range(n_cores):
        h0c = np.asarray(h0[i, 0], np.float32).reshape(2, 128)
        cols = np.stack([h0c, bz, -bz, bh], axis=-1)  # [2,128,4]
        in_maps.append({
            "x": np.ascontiguousarray(np.asarray(x[i], np.float32)),
            "wzT": wzT, "whT": whT,
            "cols": np.ascontiguousarray(cols),
            "ident": ident,
        })
    return in_maps


def kernel(x, h0, w_h_w, w_h_b, w_z_w, w_z_b):
    from concourse.bass_utils import run_bass_kernel_spmd

    nc = _get(S, 1024)
    in_maps = _make_in_maps(x, h0, w_h_w, w_h_b, w_z_w, w_z_b)
    res = run_bass_kernel_spmd(nc, in_maps, list(range(N_CORES)))
    out = np.stack([res.results[i]["out"] for i in range(N_CORES)], axis=0)
    return out.astype(np.float32)


# revision 9
# speedup vs baseline: 1.6134x; 1.6134x over previous
"""MinGRU Trainium2 kernel (nn_MinGRU_60421599920446).

Math (per batch row):
    vz[s,h] = x[s,:] @ w_z^T + bz      vh[s,h] = x[s,:] @ w_h^T + bh
    z = sigmoid(vz); h_t = (1-z_t)*h_{t-1} + z_t*vh_t   (scan over s)

Strategy: data-parallel over batch, 1 row per NeuronCore (8 cores).
Per core, work in the transposed domain [H on partitions, S on free] so the
recurrence maps onto the DVE `tensor_tensor_scan` instruction:
    state = a_t * state + b_t,  a = 1-z = sigmoid(-(vz+bz)),  b = z*(vh+bh)

fp32 matmuls on TRN2 run in LOW_HIGH (two-pass) mode with a per-matmul
4-byte LDWEIGHTS, so the matmul domain is bf16: x is cast fp32->bf16 during
the SWDGE DMA load (zero engine cost), PE 128x128 transposes run on bf16,
and the projections use bf16 weights (host-cast) with fp32 PSUM accumulate.
z/a/b and the scan state stay fp32.

Software pipeline per 1024-step s-chunk (output side lags one chunk so the
PE stream never blocks on the serial scan chain):
    gpsimd DMA: x chunk fp32->bf16 (natural [s,d]) ->
    PE transposes -> ACT copies PSUM->SBUF (x^T bf16) ->
    PE bf16 matmuls -> vz/vh PSUM fp32 ->
    ACT: z = Sigmoid(vz+bz), a = Sigmoid(-vz-bz) ->
    DVE: b = (vh + bh) * z   (scalar_tensor_tensor) ->
    DVE: tensor_tensor_scan (carry = last column of prev chunk) ->
    [next iter] PE transposes h -> ACT/DVE copies -> sync DMA out fp32.
"""

import numpy as np
from contextlib import ExitStack

B, S, D, H = 8, 8192, 256, 256
N_CORES = 8
OUT_BF16 = False   # True: scan emits bf16 (faster hT path, ~3e-3 err)

_CACHE = {}


def _build(seq_len, chunk, out_bf16):
    """Build + compile the single-core SPMD Bass program."""
    import concourse.bacc as bacc
    import concourse.tile as tile
    import concourse.mybir as mybir

    dt = mybir.dt
    f32 = dt.float32
    bf16 = dt.bfloat16
    h_dt = bf16 if out_bf16 else f32
    AF = mybir.ActivationFunctionType
    OP = mybir.AluOpType

    assert chunk % 512 == 0 and seq_len % chunk == 0
    nblk = chunk // 128          # 128-row blocks per chunk
    nchunk = seq_len // chunk

    nc = bacc.Bacc("TRN2", target_bir_lowering=False, debug=False)

    x_d = nc.dram_tensor("x", [seq_len, D], f32, kind="ExternalInput").ap()
    wzT_d = nc.dram_tensor("wzT", [D, H], bf16, kind="ExternalInput").ap()
    whT_d = nc.dram_tensor("whT", [D, H], bf16, kind="ExternalInput").ap()
    # packed per-partition columns: [half m][128][h0, bz, -bz, bh]
    cols_d = nc.dram_tensor("cols", [2, 128, 4], f32, kind="ExternalInput").ap()
    idb_d = nc.dram_tensor("identb", [128, 128], bf16, kind="ExternalInput").ap()
    idf_d = nc.dram_tensor("identf", [128, 128], f32, kind="ExternalInput").ap()
    out_d = nc.dram_tensor("out", [seq_len, H], f32, kind="ExternalOutput").ap()

    # chunked views: [chunk-idx, partition(s within block), block, feature]
    x_v = x_d.rearrange("(c t p) d -> c p t d", p=128, t=nblk)
    out_v = out_d.rearrange("(c t p) h -> c p t h", p=128, t=nblk)

    with tile.TileContext(nc) as tc, ExitStack() as ctx:
        const = ctx.enter_context(tc.tile_pool(name="const", bufs=1))
        xin = ctx.enter_context(tc.tile_pool(name="xin", bufs=3))
        xTp = ctx.enter_context(tc.tile_pool(name="xT", bufs=2))
        zp = ctx.enter_context(tc.tile_pool(name="z", bufs=2))
        ap_ = ctx.enter_context(tc.tile_pool(name="a", bufs=2))
        bp = ctx.enter_context(tc.tile_pool(name="b", bufs=2))
        hp = ctx.enter_context(tc.tile_pool(name="h", bufs=3))
        hop = ctx.enter_context(tc.tile_pool(name="ho", bufs=2))
        vzp = ctx.enter_context(tc.tile_pool(name="vz", bufs=2, space="PSUM"))
        vhp = ctx.enter_context(tc.tile_pool(name="vh", bufs=2, space="PSUM"))
        trp = ctx.enter_context(tc.tile_pool(name="tr", bufs=2, space="PSUM"))

        identb = const.tile([128, 128], bf16, tag="identb")
        nc.sync.dma_start(identb[:], idb_d[:, :])
        ident_h = identb
        if not out_bf16:
            identf = const.tile([128, 128], f32, tag="identf")
            nc.sync.dma_start(identf[:], idf_d[:, :])
            ident_h = identf
        cols = []
        for m in range(2):
            t = const.tile([128, 4], f32, tag=f"cols{m}")
            nc.sync.dma_start(t[:], cols_d[m])
            cols.append(t)
        wzT, whT = [], []
        for k in range(2):
            tz = const.tile([128, H], bf16, tag=f"wz{k}")
            nc.sync.dma_start(tz[:], wzT_d[k * 128:(k + 1) * 128, :])
            wzT.append(tz)
            th = const.tile([128, H], bf16, tag=f"wh{k}")
            nc.sync.dma_start(th[:], whT_d[k * 128:(k + 1) * 128, :])
            whT.append(th)

        h_hist = {}

        def emit_input_side(c):
            """DMA-in, x-transpose, projections, z/a/b, scan for chunk c."""
            xn = xin.tile([128, nblk * D], bf16, tag="xn", name="xn")
            nc.gpsimd.dma_start(          # SWDGE: casts fp32 -> bf16
                xn[:].rearrange("p (t d) -> p t d", d=D), x_v[c])

            xT = [xTp.tile([128, chunk], bf16, tag=f"xt{k}", name=f"xt{k}")
                  for k in range(2)]
            for k in range(2):
                for g in range(chunk // 512):
                    pt = trp.tile([128, 512], bf16, tag="tr", name="ptx")
                    for j in range(4):
                        t = g * 4 + j
                        nc.tensor.transpose(
                            pt[:, j * 128:(j + 1) * 128],
                            xn[:, t * D + k * 128: t * D + (k + 1) * 128],
                            identb[:],
                        )
                    nc.scalar.copy(xT[k][:, g * 512:(g + 1) * 512], pt[:])

            vz = [vzp.tile([128, chunk], f32, tag="vz", name=f"vz{m}")
                  for m in range(2)]
            for m in range(2):
                for k in range(2):
                    for s2 in range(chunk // 512):
                        nc.tensor.matmul(
                            vz[m][:, s2 * 512:(s2 + 1) * 512],
                            wzT[k][:, m * 128:(m + 1) * 128],
                            xT[k][:, s2 * 512:(s2 + 1) * 512],
                            start=(k == 0), stop=(k == 1),
                        )
            z = [zp.tile([128, chunk], f32, tag=f"z{m}", name=f"z{m}")
                 for m in range(2)]
            a = [ap_.tile([128, chunk], f32, tag=f"a{m}", name=f"a{m}")
                 for m in range(2)]
            for m in range(2):
                nc.scalar.activation(z[m][:], vz[m][:], AF.Sigmoid,
                                     bias=cols[m][:, 1:2], scale=1.0)
                nc.scalar.activation(a[m][:], vz[m][:], AF.Sigmoid,
                                     bias=cols[m][:, 2:3], scale=-1.0)

            b = [bp.tile([128, chunk], f32, tag=f"b{m}", name=f"b{m}")
                 for m in range(2)]
            for m in range(2):
                for s2 in range(chunk // 512):
                    vht = vhp.tile([128, 512], f32, tag="vh", name="vht")
                    for k in range(2):
                        nc.tensor.matmul(
                            vht[:],
                            whT[k][:, m * 128:(m + 1) * 128],
                            xT[k][:, s2 * 512:(s2 + 1) * 512],
                            start=(k == 0), stop=(k == 1),
                        )
                    nc.vector.scalar_tensor_tensor(
                        b[m][:, s2 * 512:(s2 + 1) * 512],
                        vht[:], cols[m][:, 3:4],
                        z[m][:, s2 * 512:(s2 + 1) * 512],
                        op0=OP.add, op1=OP.mult,
                    )

            h = [hp.tile([128, chunk], h_dt, tag=f"h{m}", name=f"h{m}")
                 for m in range(2)]
            for m in range(2):
                init = (cols[m][:, 0:1] if c == 0
                        else h_hist[c - 1][m][:, chunk - 1:chunk])
                nc.vector.tensor_tensor_scan(
                    h[m][:], a[m][:], b[m][:], init,
                    op0=OP.mult, op1=OP.add,
                )
            h_hist[c] = h

        def emit_output_side(c):
            """h-transpose back to natural [s, h] + store for chunk c."""
            h = h_hist[c]
            ho = hop.tile([128, nblk * H], f32, tag="ho", name="ho")
            for g in range(chunk // 256):
                pt = trp.tile([128, 512], h_dt, tag="tr", name="pth")
                for j in range(2):
                    t = g * 2 + j
                    for m in range(2):
                        nc.tensor.transpose(
                            pt[:, j * 256 + m * 128: j * 256 + (m + 1) * 128],
                            h[m][:, t * 128:(t + 1) * 128],
                            ident_h[:],
                        )
                # split the PSUM->SBUF copies between ACT and DVE
                if g % 2 == 0:
                    nc.scalar.copy(ho[:, g * 512:(g + 1) * 512], pt[:])
                else:
                    nc.vector.tensor_copy(ho[:, g * 512:(g + 1) * 512], pt[:])
            nc.sync.dma_start(out_v[c],
                              ho[:].rearrange("p (t h) -> p t h", h=H))
            del h_hist[c]

        for c in range(nchunk + 1):
            if c < nchunk:
                emit_input_side(c)
            if c >= 1:
                emit_output_side(c - 1)

    nc.compile()
    return nc


def _get(seq_len, chunk, out_bf16=OUT_BF16):
    key = (seq_len, chunk, out_bf16)
    if key not in _CACHE:
        _CACHE[key] = _build(seq_len, chunk, out_bf16)
    return _CACHE[key]


def _make_in_maps(x, h0, w_h_w, w_h_b, w_z_w, w_z_b, n_cores=N_CORES):
    import ml_dtypes
    bf16 = ml_dtypes.bfloat16
    wzT = np.ascontiguousarray(np.asarray(w_z_w, np.float32).T.astype(bf16))
    whT = np.ascontiguousarray(np.asarray(w_h_w, np.float32).T.astype(bf16))
    bz = np.asarray(w_z_b, np.float32).reshape(2, 128)
    bh = np.asarray(w_h_b, np.float32).reshape(2, 128)
    identf = np.eye(128, dtype=np.float32)
    identb = identf.astype(bf16)
    in_maps = []
    for i in range(n_cores):
        h0c = np.asarray(h0[i, 0], np.float32).reshape(2, 128)
        cols = np.stack([h0c, bz, -bz, bh], axis=-1)  # [2,128,4]
        in_maps.append({
            "x": np.ascontiguousarray(np.asarray(x[i], np.float32)),
            "wzT": wzT, "whT": whT,
            "cols": np.ascontiguousarray(cols),
            "identb": identb, "identf": identf,
        })
    return in_maps


def kernel(x, h0, w_h_w, w_h_b, w_z_w, w_z_b):
    from concourse.bass_utils import run_bass_kernel_spmd

    nc = _get(S, 1024)
    in_maps = _make_in_maps(x, h0, w_h_w, w_h_b, w_z_w, w_z_b)
    res = run_bass_kernel_spmd(nc, in_maps, list(range(N_CORES)))
    out = np.stack([res.results[i]["out"] for i in range(N_CORES)], axis=0)
    return out.astype(np.float32)


# revision 12
# speedup vs baseline: 1.6181x; 1.0029x over previous
"""MinGRU Trainium2 kernel (nn_MinGRU_60421599920446).

Math (per batch row):
    vz[s,h] = x[s,:] @ w_z^T + bz      vh[s,h] = x[s,:] @ w_h^T + bh
    z = sigmoid(vz); h_t = (1-z_t)*h_{t-1} + z_t*vh_t   (scan over s)

Strategy: data-parallel over batch, 1 row per NeuronCore (8 cores).
Per core, work in the transposed domain [H on partitions, S on free] so the
recurrence maps onto the DVE `tensor_tensor_scan` instruction:
    state = a_t * state + b_t,  a = 1-z = sigmoid(-(vz+bz)),  b = z*(vh+bh)

fp32 matmuls on TRN2 run in LOW_HIGH (two-pass) mode with a per-matmul
4-byte LDWEIGHTS, so the matmul domain is bf16: x is cast fp32->bf16 during
the SWDGE DMA load (zero engine cost), PE 128x128 transposes run on bf16,
and the projections use bf16 weights (host-cast) with fp32 PSUM accumulate.
z/a/b and the scan state stay fp32.

Software pipeline per 1024-step s-chunk (output side lags one chunk so the
PE stream never blocks on the serial scan chain):
    gpsimd DMA: x chunk fp32->bf16 (natural [s,d]) ->
    PE transposes -> ACT copies PSUM->SBUF (x^T bf16) ->
    PE bf16 matmuls -> vz/vh PSUM fp32 ->
    ACT: z = Sigmoid(vz+bz), a = Sigmoid(-vz-bz) ->
    DVE: b = (vh + bh) * z   (scalar_tensor_tensor) ->
    DVE: tensor_tensor_scan (carry = last column of prev chunk) ->
    [next iter] PE transposes h -> ACT/DVE copies -> sync DMA out fp32.
"""

import numpy as np
from contextlib import ExitStack

B, S, D, H = 8, 8192, 256, 256
N_CORES = 8
OUT_BF16 = True    # True: scan emits bf16 (faster hT path, ~3e-3 err)

_CACHE = {}


def _build(seq_len, chunk, out_bf16):
    """Build + compile the single-core SPMD Bass program."""
    import concourse.bacc as bacc
    import concourse.tile as tile
    import concourse.mybir as mybir

    dt = mybir.dt
    f32 = dt.float32
    bf16 = dt.bfloat16
    h_dt = bf16 if out_bf16 else f32
    AF = mybir.ActivationFunctionType
    OP = mybir.AluOpType

    assert chunk % 512 == 0 and seq_len % chunk == 0
    nblk = chunk // 128          # 128-row blocks per chunk
    nchunk = seq_len // chunk

    nc = bacc.Bacc("TRN2", target_bir_lowering=False, debug=False)

    x_d = nc.dram_tensor("x", [seq_len, D], f32, kind="ExternalInput").ap()
    wzT_d = nc.dram_tensor("wzT", [D, H], bf16, kind="ExternalInput").ap()
    whT_d = nc.dram_tensor("whT", [D, H], bf16, kind="ExternalInput").ap()
    # packed per-partition columns: [half m][128][h0, bz, -bz, bh]
    cols_d = nc.dram_tensor("cols", [2, 128, 4], f32, kind="ExternalInput").ap()
    idb_d = nc.dram_tensor("identb", [128, 128], bf16, kind="ExternalInput").ap()
    idf_d = nc.dram_tensor("identf", [128, 128], f32, kind="ExternalInput").ap()
    out_d = nc.dram_tensor("out", [seq_len, H], f32, kind="ExternalOutput").ap()

    # chunked views: [chunk-idx, partition(s within block), block, feature]
    x_v = x_d.rearrange("(c t p) d -> c p t d", p=128, t=nblk)
    out_v = out_d.rearrange("(c t p) h -> c p t h", p=128, t=nblk)

    with tile.TileContext(nc) as tc, ExitStack() as ctx:
        const = ctx.enter_context(tc.tile_pool(name="const", bufs=1))
        xin = ctx.enter_context(tc.tile_pool(name="xin", bufs=3))
        xTp = ctx.enter_context(tc.tile_pool(name="xT", bufs=3))
        zp = ctx.enter_context(tc.tile_pool(name="z", bufs=2))
        ap_ = ctx.enter_context(tc.tile_pool(name="a", bufs=2))
        bp = ctx.enter_context(tc.tile_pool(name="b", bufs=2))
        hp = ctx.enter_context(tc.tile_pool(name="h", bufs=3))
        hop = ctx.enter_context(tc.tile_pool(name="ho", bufs=3))
        vzp = ctx.enter_context(tc.tile_pool(name="vz", bufs=2, space="PSUM"))
        vhp = ctx.enter_context(tc.tile_pool(name="vh", bufs=2, space="PSUM"))
        trp = ctx.enter_context(tc.tile_pool(name="tr", bufs=2, space="PSUM"))

        identb = const.tile([128, 128], bf16, tag="identb")
        nc.sync.dma_start(identb[:], idb_d[:, :])
        ident_h = identb
        if not out_bf16:
            identf = const.tile([128, 128], f32, tag="identf")
            nc.sync.dma_start(identf[:], idf_d[:, :])
            ident_h = identf
        cols = []
        for m in range(2):
            t = const.tile([128, 4], f32, tag=f"cols{m}")
            nc.sync.dma_start(t[:], cols_d[m])
            cols.append(t)
        wzT, whT = [], []
        for k in range(2):
            tz = const.tile([128, H], bf16, tag=f"wz{k}")
            nc.sync.dma_start(tz[:], wzT_d[k * 128:(k + 1) * 128, :])
            wzT.append(tz)
            th = const.tile([128, H], bf16, tag=f"wh{k}")
            nc.sync.dma_start(th[:], whT_d[k * 128:(k + 1) * 128, :])
            whT.append(th)

        h_hist = {}

        def emit_input_side(c):
            """DMA-in, x-transpose, projections, z/a/b, scan for chunk c."""
            xn = xin.tile([128, nblk * D], bf16, tag="xn", name="xn")
            nc.gpsimd.dma_start(          # SWDGE: casts fp32 -> bf16
                xn[:].rearrange("p (t d) -> p t d", d=D), x_v[c])

            xT = [xTp.tile([128, chunk], bf16, tag=f"xt{k}", name=f"xt{k}")
                  for k in range(2)]
            for k in range(2):
                for g in range(chunk // 512):
                    pt = trp.tile([128, 512], bf16, tag="tr", name="ptx")
                    for j in range(4):
                        t = g * 4 + j
                        nc.tensor.transpose(
                            pt[:, j * 128:(j + 1) * 128],
                            xn[:, t * D + k * 128: t * D + (k + 1) * 128],
                            identb[:],
                        )
                    nc.scalar.copy(xT[k][:, g * 512:(g + 1) * 512], pt[:])

            vz = [vzp.tile([128, chunk], f32, tag="vz", name=f"vz{m}")
                  for m in range(2)]
            for m in range(2):
                for k in range(2):
                    for s2 in range(chunk // 512):
                        nc.tensor.matmul(
                            vz[m][:, s2 * 512:(s2 + 1) * 512],
                            wzT[k][:, m * 128:(m + 1) * 128],
                            xT[k][:, s2 * 512:(s2 + 1) * 512],
                            start=(k == 0), stop=(k == 1),
                        )
            z = [zp.tile([128, chunk], f32, tag=f"z{m}", name=f"z{m}")
                 for m in range(2)]
            a = [ap_.tile([128, chunk], f32, tag=f"a{m}", name=f"a{m}")
                 for m in range(2)]
            for m in range(2):
                nc.scalar.activation(z[m][:], vz[m][:], AF.Sigmoid,
                                     bias=cols[m][:, 1:2], scale=1.0)
                nc.scalar.activation(a[m][:], vz[m][:], AF.Sigmoid,
                                     bias=cols[m][:, 2:3], scale=-1.0)

            b = [bp.tile([128, chunk], f32, tag=f"b{m}", name=f"b{m}")
                 for m in range(2)]
            for m in range(2):
                for s2 in range(chunk // 512):
                    vht = vhp.tile([128, 512], f32, tag="vh", name="vht")
                    for k in range(2):
                        nc.tensor.matmul(
                            vht[:],
                            whT[k][:, m * 128:(m + 1) * 128],
                            xT[k][:, s2 * 512:(s2 + 1) * 512],
                            start=(k == 0), stop=(k == 1),
                        )
                    nc.vector.scalar_tensor_tensor(
                        b[m][:, s2 * 512:(s2 + 1) * 512],
                        vht[:], cols[m][:, 3:4],
                        z[m][:, s2 * 512:(s2 + 1) * 512],
                        op0=OP.add, op1=OP.mult,
                    )

            h = [hp.tile([128, chunk], h_dt, tag=f"h{m}", name=f"h{m}")
                 for m in range(2)]
            for m in range(2):
                init = (cols[m][:, 0:1] if c == 0
                        else h_hist[c - 1][m][:, chunk - 1:chunk])
                nc.vector.tensor_tensor_scan(
                    h[m][:], a[m][:], b[m][:], init,
                    op0=OP.mult, op1=OP.add,
                )
            h_hist[c] = h

        def emit_output_side(c):
            """h-transpose back to natural [s, h] + store for chunk c."""
            h = h_hist[c]
            ho = hop.tile([128, nblk * H], f32, tag="ho", name="ho")
            for g in range(chunk // 256):
                pt = trp.tile([128, 512], h_dt, tag="tr", name="pth")
                for j in range(2):
                    t = g * 2 + j
                    for m in range(2):
                        nc.tensor.transpose(
                            pt[:, j * 256 + m * 128: j * 256 + (m + 1) * 128],
                            h[m][:, t * 128:(t + 1) * 128],
                            ident_h[:],
                        )
                # split the PSUM->SBUF copies between ACT and DVE
                if g % 2 == 0:
                    nc.scalar.copy(ho[:, g * 512:(g + 1) * 512], pt[:])
                else:
                    nc.vector.tensor_copy(ho[:, g * 512:(g + 1) * 512], pt[:])
            nc.sync.dma_start(out_v[c],
                              ho[:].rearrange("p (t h) -> p t h", h=H))
            del h_hist[c]

        for c in range(nchunk + 1):
            if c < nchunk:
                emit_input_side(c)
            if c >= 1:
                emit_output_side(c - 1)

    nc.compile()
    return nc


def _get(seq_len, chunk, out_bf16=OUT_BF16):
    key = (seq_len, chunk, out_bf16)
    if key not in _CACHE:
        _CACHE[key] = _build(seq_len, chunk, out_bf16)
    return _CACHE[key]


def _make_in_maps(x, h0, w_h_w, w_h_b, w_z_w, w_z_b, n_cores=N_CORES):
    import ml_dtypes
    bf16 = ml_dtypes.bfloat16
    wzT = np.ascontiguousarray(np.asarray(w_z_w, np.float32).T.astype(bf16))
    whT = np.ascontiguousarray(np.asarray(w_h_w, np.float32).T.astype(bf16))
    bz = np.asarray(w_z_b, np.float32).reshape(2, 128)
    bh = np.asarray(w_h_b, np.float32).reshape(2, 128)
    identf = np.eye(128, dtype=np.float32)
    identb = identf.astype(bf16)
    in_maps = []
    for i in range(n_cores):
        h0c = np.asarray(h0[i, 0], np.float32).reshape(2, 128)
        cols = np.stack([h0c, bz, -bz, bh], axis=-1)  # [2,128,4]
        in_maps.append({
            "x": np.ascontiguousarray(np.asarray(x[i], np.float32)),
            "wzT": wzT, "whT": whT,
            "cols": np.ascontiguousarray(cols),
            "identb": identb, "identf": identf,
        })
    return in_maps


def kernel(x, h0, w_h_w, w_h_b, w_z_w, w_z_b):
    from concourse.bass_utils import run_bass_kernel_spmd

    nc = _get(S, 1024)
    in_maps = _make_in_maps(x, h0, w_h_w, w_h_b, w_z_w, w_z_b)
    res = run_bass_kernel_spmd(nc, in_maps, list(range(N_CORES)))
    out = np.stack([res.results[i]["out"] for i in range(N_CORES)], axis=0)
    return out.astype(np.float32)
